# revision 4
# baseline (speedup 1.0000x reference)
"""Depthwise 4x4 blur (upfirdn2d pad=(2,1)) on TRN2, 8 NeuronCores.

The 2D blur kernel [1,3,3,1]x[1,3,3,1]/64 is separable, so
out = Av @ X @ Ah^T per image, where Av/Ah are 128x128 banded matrices
(4-tap band; H/W zero-padding folded into the band clipping). On the PE:

  pass 1:  tmpT = lhsT.T @ rhs with lhsT = X (the image as the STATIONARY
           operand), rhs = Av^T          -> tmpT = (Av @ X)^T   [w, h] PSUM
  pass 2:  outT = lhsT.T @ rhs with lhsT = Ah^T, rhs = tmpT (4 images)
                                         -> outT = (Av@X@Ah^T)^T [w, h] PSUM

Putting the per-image matrix on the stationary side in pass 1 means NO
transposes anywhere: the output simply leaves the device W-major and the
host untransposes for free. PE streams 256 cols/image (vs 1024 for the
4-banded-matmul hilo formulation) plus one 128-col LDWEIGHTS per image
(FWL, overlapped with the previous matmul via the background weight
buffer).

Everything on-chip is fp16 (PSUM accumulation stays fp32): rel err ~7e-4
vs the 2e-2 gate, and HBM traffic halves vs f32 (16.8 MB/core total).
Host pre-arranges x as [H, C, W] fp16 so every DMA row is a contiguous
4 KB per partition (the f32 baseline's 524 B rows capped each HWDGE ring
at ~190 GB/s). Input DMAs ride the SP HWDGE ring, output DMAs the
GpSimd SWDGE path, leaving ACT/DVE free for PSUM evacuation.

Sharding: batch dim (8 batches -> 8 cores), 256 images of 128x128 each.
"""

import numpy as np
from contextlib import ExitStack

import concourse.bass as bass
import concourse.bacc as bacc
import concourse.tile as tile
import concourse.mybir as mybir
from concourse.bass_utils import run_bass_kernel_spmd

N_CORES = 8
B, C, H, W = 8, 256, 128, 128
GROUP = 4          # images per pass-2 matmul / PSUM bank (4*128 = 512 f32)
PAIR = 8           # images per PSUM tile (2 banks) / per PSUM->SBUF copy
SUPER = 32         # images per DMA (1.05 MB transfers)
MODE = "sep16"

F32 = mybir.dt.float32
F16 = mybir.dt.float16


def _body_sep16(ctx, tc, o_ap, x_ap, w_ap, ramp=True, out_eng="gpsimd"):
    nc = tc.nc
    wpool = ctx.enter_context(tc.tile_pool(name="wts", bufs=1))
    xpool = ctx.enter_context(tc.tile_pool(name="xin", bufs=4))
    tpool = ctx.enter_context(tc.tile_pool(name="tmid", bufs=6))
    opool = ctx.enter_context(tc.tile_pool(name="oup", bufs=3))
    p1pool = ctx.enter_context(tc.tile_pool(name="ps1", bufs=2, space="PSUM"))
    p2pool = ctx.enter_context(tc.tile_pool(name="ps2", bufs=2, space="PSUM"))

    wt = wpool.tile([H, 2 * H], F16)
    nc.scalar.dma_start(wt[:], w_ap)
    wv = wt[:, :H]     # Av^T: moving operand of pass 1
    wh = wt[:, H:]     # Ah^T: stationary operand of pass 2

    # small supertiles at the ends prime/drain the DMA->PE->copy->DMA pipe
    if ramp:
        sizes = [8, 8, 16] + [SUPER] * ((C - 64) // SUPER) + [16, 8, 8]
    else:
        sizes = [SUPER] * (C // SUPER)
    assert sum(sizes) == C

    oeng = {"gpsimd": nc.gpsimd, "scalar": nc.scalar, "sync": nc.sync}[out_eng]
    c0 = 0
    for sz in sizes:
        xt = xpool.tile([H, sz * W], F16, tag="xt")
        nc.sync.dma_start(
            xt[:].rearrange("h (c w) -> h c w", c=sz), x_ap[:, c0 : c0 + sz]
        )
        ot = opool.tile([H, sz * H], F16, tag="ot")
        for p0 in range(0, sz, PAIR):
            pc = min(PAIR, sz - p0)
            # pass 1: per-image stationary, 2-bank PSUM tile (8 images)
            pt1 = p1pool.tile([H, pc * H], F32, tag="pt1")
            for i in range(pc):
                c = p0 + i
                nc.tensor.matmul(
                    pt1[:, i * H : (i + 1) * H],
                    xt[:, c * W : (c + 1) * W],
                    wv,
                    start=True,
                    stop=True,
                )
            tt = tpool.tile([H, pc * H], F16, tag="tt")
            nc.vector.tensor_copy(tt[:], pt1[:])
            # pass 2: fixed stationary, one bank (4 images) per matmul
            pt2 = p2pool.tile([H, pc * H], F32, tag="pt2")
            for g in range(0, pc, GROUP):
                ge = min(g + GROUP, pc)
                nc.tensor.matmul(
                    pt2[:, g * H : ge * H],
                    wh,
                    tt[:, g * H : ge * H],
                    start=True,
                    stop=True,
                )
            nc.scalar.copy(ot[:, (p0 + 0) * H : (p0 + pc) * H], pt2[:])
        oeng.dma_start(
            o_ap[:, c0 : c0 + sz], ot[:].rearrange("w (c h) -> w c h", c=sz)
        )
        c0 += sz


def build_module(mode=MODE, **kw):
    nc = bacc.Bacc(
        "TRN2", target_bir_lowering=False, debug=False, num_devices=N_CORES
    )
    x_ap = nc.dram_tensor("x", [H, C, W], F16, kind="ExternalInput").ap()
    w_ap = nc.dram_tensor("wts", [H, 2 * H], F16, kind="ExternalInput").ap()
    o_ap = nc.dram_tensor("out", [W, C, H], F16, kind="ExternalOutput").ap()
    with tile.TileContext(nc) as tc:
        with ExitStack() as ctx:
            _body_sep16(ctx, tc, o_ap, x_ap, w_ap, **kw)
    nc.compile()
    return nc


def band_mat(taps):
    """A[h, h+i-2] = taps[::-1][i], rows/cols clipped to [0,128)."""
    kf = np.asarray(taps, np.float32)[::-1]
    A = np.zeros((H, H), np.float32)
    for i in range(len(kf)):
        d = i - 2
        h0, h1 = max(0, -d), min(H, H - d)
        idx = np.arange(h0, h1)
        A[idx, idx + d] = kf[i]
    return A


_module_cache = {}


def _get_module(mode=MODE, **kw):
    key = (mode, tuple(sorted(kw.items())))
    if key not in _module_cache:
        _module_cache[key] = build_module(mode, **kw)
    return _module_cache[key]


def kernel(x, kernel, _trace=False, _trace_kwargs=None, _mode=None, _build_kw=None):
    x = np.asarray(x)
    assert x.shape == (B, C, H, W), x.shape
    k2d = np.asarray(kernel, np.float32)
    # rank-1 factorization of the (sum-normalized) separable 2D kernel
    av = k2d.sum(1)
    ah = k2d.sum(0) / k2d.sum()
    wts = np.concatenate(
        [band_mat(av).T, band_mat(ah).T], axis=1
    ).astype(np.float16)
    xT = x.transpose(0, 2, 1, 3).astype(np.float16)  # [B, H, C, W] contiguous
    nc = _get_module(_mode or MODE, **(_build_kw or {}))
    in_maps = [{"x": xT[i], "wts": wts} for i in range(N_CORES)]
    res = run_bass_kernel_spmd(
        nc, in_maps, list(range(N_CORES)), trace=_trace, **(_trace_kwargs or {})
    )
    out = np.stack([res.results[i]["out"] for i in range(N_CORES)], axis=0)
    out = out.transpose(0, 2, 3, 1).astype(np.float32)  # [B,W,C,H]->[B,C,H,W]
    if _trace:
        return out, res
    return out


# revision 7
# speedup vs baseline: 1.1558x; 1.1558x over previous
"""Depthwise 4x4 blur (upfirdn2d pad=(2,1)) on TRN2, 8 NeuronCores.

The 2D blur kernel [1,3,3,1]x[1,3,3,1]/64 is separable, so
out = Av @ X @ Ah^T per image, where Av/Ah are 128x128 banded matrices
(4-tap band; H/W zero-padding folded into the band clipping). On the PE:

  pass 1:  tmpT = lhsT.T @ rhs with lhsT = X (the image as the STATIONARY
           operand), rhs = Av^T          -> tmpT = (Av @ X)^T   [w, h] PSUM
  pass 2:  outT = lhsT.T @ rhs with lhsT = Ah^T, rhs = tmpT (4 images)
                                         -> outT = (Av@X@Ah^T)^T [w, h] PSUM

Putting the per-image matrix on the stationary side in pass 1 means NO
transposes anywhere: the output simply leaves the device W-major and the
host untransposes for free. PE streams 256 cols/image (vs 1024 for the
4-banded-matmul hilo formulation) plus one 128-col LDWEIGHTS per image
(FWL, overlapped with the previous matmul via the background weight
buffer).

Everything on-chip is fp16 (PSUM accumulation stays fp32): rel err ~7e-4
vs the 2e-2 gate, and HBM traffic halves vs f32 (16.8 MB/core total).
Host pre-arranges x as [H, C, W] fp16 so every DMA row is a contiguous
4 KB per partition (the f32 baseline's 524 B rows capped each HWDGE ring
at ~190 GB/s). Input DMAs ride the SP HWDGE ring, output DMAs the
GpSimd SWDGE path, leaving ACT/DVE free for PSUM evacuation.

Sharding: batch dim (8 batches -> 8 cores), 256 images of 128x128 each.
"""

import numpy as np
from contextlib import ExitStack

import concourse.bass as bass
import concourse.bacc as bacc
import concourse.tile as tile
import concourse.mybir as mybir
from concourse.bass_utils import run_bass_kernel_spmd

N_CORES = 8
B, C, H, W = 8, 256, 128, 128
GROUP = 4          # images per pass-2 matmul / PSUM bank (4*128 = 512 f32)
PAIR = 8           # images per pass-2 PSUM tile (2 banks) / ACT copy
SUPER = 16         # images per DMA (524 KB transfers)
MODE = "sep16"

F32 = mybir.dt.float32
F16 = mybir.dt.float16


def _body_sep16(ctx, tc, o_ap, x_ap, w_ap, ramp=True, out_eng="gpsimd"):
    nc = tc.nc
    wpool = ctx.enter_context(tc.tile_pool(name="wts", bufs=1))
    # deep input prefetch: the whole fp16 input fits in SBUF, so let the
    # input ring run back-to-back instead of throttling on compute
    xpool = ctx.enter_context(tc.tile_pool(name="xin", bufs=16))
    tpool = ctx.enter_context(tc.tile_pool(name="tmid", bufs=8))
    opool = ctx.enter_context(tc.tile_pool(name="oup", bufs=4))
    p1pool = ctx.enter_context(tc.tile_pool(name="ps1", bufs=4, space="PSUM"))
    p2pool = ctx.enter_context(tc.tile_pool(name="ps2", bufs=2, space="PSUM"))

    wt = wpool.tile([H, 2 * H], F16)
    nc.scalar.dma_start(wt[:], w_ap)
    wv = wt[:, :H]     # Av^T: moving operand of pass 1
    wh = wt[:, H:]     # Ah^T: stationary operand of pass 2

    # small supertiles at the ends prime/drain the DMA->PE->copy->DMA pipe
    if ramp:
        sizes = [8, 8] + [SUPER] * ((C - 32) // SUPER) + [8, 8]
    else:
        sizes = [SUPER] * (C // SUPER)
    assert sum(sizes) == C

    oeng = {"gpsimd": nc.gpsimd, "scalar": nc.scalar, "sync": nc.sync}[out_eng]
    c0 = 0
    for sz in sizes:
        xt = xpool.tile([H, sz * W], F16, tag="xt")
        nc.sync.dma_start(
            xt[:].rearrange("h (c w) -> h c w", c=sz), x_ap[:, c0 : c0 + sz]
        )
        ot = opool.tile([H, sz * H], F16, tag="ot")
        for p0 in range(0, sz, PAIR):
            pc = min(PAIR, sz - p0)
            # pass 1: per-image stationary; 1-bank PSUM groups, DVE copies
            # (DVE 2-bank copies are slower than 2x 1-bank; ACT is opposite)
            tts = []
            for g in range(p0, p0 + pc, GROUP):
                gc = min(GROUP, p0 + pc - g)
                pt1 = p1pool.tile([H, gc * H], F32, tag="pt1")
                for i in range(gc):
                    c = g + i
                    nc.tensor.matmul(
                        pt1[:, i * H : (i + 1) * H],
                        xt[:, c * W : (c + 1) * W],
                        wv,
                        start=True,
                        stop=True,
                    )
                tt = tpool.tile([H, gc * H], F16, tag="tt")
                nc.vector.tensor_copy(tt[:], pt1[:])
                tts.append((tt, gc))
            # pass 2: fixed stationary, 2-bank PSUM tile, one ACT copy
            pt2 = p2pool.tile([H, pc * H], F32, tag="pt2")
            o = 0
            for tt, gc in tts:
                nc.tensor.matmul(
                    pt2[:, o * H : (o + gc) * H],
                    wh,
                    tt[:],
                    start=True,
                    stop=True,
                )
                o += gc
            nc.scalar.copy(ot[:, p0 * H : (p0 + pc) * H], pt2[:])
        oeng.dma_start(
            o_ap[:, c0 : c0 + sz], ot[:].rearrange("w (c h) -> w c h", c=sz)
        )
        c0 += sz


def build_module(mode=MODE, **kw):
    nc = bacc.Bacc(
        "TRN2", target_bir_lowering=False, debug=False, num_devices=N_CORES
    )
    x_ap = nc.dram_tensor("x", [H, C, W], F16, kind="ExternalInput").ap()
    w_ap = nc.dram_tensor("wts", [H, 2 * H], F16, kind="ExternalInput").ap()
    o_ap = nc.dram_tensor("out", [W, C, H], F16, kind="ExternalOutput").ap()
    with tile.TileContext(nc) as tc:
        with ExitStack() as ctx:
            _body_sep16(ctx, tc, o_ap, x_ap, w_ap, **kw)
    nc.compile()
    return nc


def band_mat(taps):
    """A[h, h+i-2] = taps[::-1][i], rows/cols clipped to [0,128)."""
    kf = np.asarray(taps, np.float32)[::-1]
    A = np.zeros((H, H), np.float32)
    for i in range(len(kf)):
        d = i - 2
        h0, h1 = max(0, -d), min(H, H - d)
        idx = np.arange(h0, h1)
        A[idx, idx + d] = kf[i]
    return A


_module_cache = {}


def _get_module(mode=MODE, **kw):
    key = (mode, tuple(sorted(kw.items())))
    if key not in _module_cache:
        _module_cache[key] = build_module(mode, **kw)
    return _module_cache[key]


def kernel(x, kernel, _trace=False, _trace_kwargs=None, _mode=None, _build_kw=None):
    x = np.asarray(x)
    assert x.shape == (B, C, H, W), x.shape
    k2d = np.asarray(kernel, np.float32)
    # rank-1 factorization of the (sum-normalized) separable 2D kernel
    av = k2d.sum(1)
    ah = k2d.sum(0) / k2d.sum()
    wts = np.concatenate(
        [band_mat(av).T, band_mat(ah).T], axis=1
    ).astype(np.float16)
    xT = x.transpose(0, 2, 1, 3).astype(np.float16)  # [B, H, C, W] contiguous
    nc = _get_module(_mode or MODE, **(_build_kw or {}))
    in_maps = [{"x": xT[i], "wts": wts} for i in range(N_CORES)]
    res = run_bass_kernel_spmd(
        nc, in_maps, list(range(N_CORES)), trace=_trace, **(_trace_kwargs or {})
    )
    out = np.stack([res.results[i]["out"] for i in range(N_CORES)], axis=0)
    out = out.transpose(0, 2, 3, 1).astype(np.float32)  # [B,W,C,H]->[B,C,H,W]
    if _trace:
        return out, res
    return out
